# revision 29
# baseline (speedup 1.0000x reference)
"""Causal self-attention (B=4, T=2048, D=1024, H=16) on 8 trn2 NeuronCores.

Sharding: 2-D data x tensor parallel. Core c handles batch b = c//2 and
head group hg = c%2 (8 of the 16 heads). Each core computes its 8 heads'
qkv projection, causal attention, and a partial output projection
(columns of w_out for its heads); the host sums the two partials per
batch element and adds b_out.

On-device dataflow is fully transposed ([feature, token] layouts) so no
transposes are ever materialized:
  - qT/kT      = (w_q|w_k) @ x^T          (f32r matmuls, fp32 precision)
  - v_nat      = x @ w_v^T + b_v          (natural [token, feat] = PV lhsT)
  - S^T tiles  = matmul(lhsT=kT[64,128], rhs=qT[64,512]); two heads packed
                 into PE rows 0-63 / 64-127 via tile_position
  - P = exp(S^T * 0.125) with causal narrowing + 128x128 triangle mask
  - O^T        = matmul(lhsT=v_aug[128,65], rhs=P[128,512]); 65th column of
                 v_aug is ones => PSUM row 64 = softmax denominator
  - normalize via DVE mult with partition-broadcast reciprocal
  - out^T      = w_outT-contract over attention features (bf16)

Engine assignment: ACT = exp + denominator staging only; DVE = psum->sbuf
copies, masks, normalize; GpSimd = partition broadcasts; PE stays dense
(warm-up matmuls cover the initial DMA window to hold the HAM clock at
2.4 GHz).
"""
import sys

import numpy as np

if "/opt/trn_rl_repo" not in sys.path:
    sys.path.insert(0, "/opt/trn_rl_repo")

import ml_dtypes

D = 1024          # d_model
T = 2048          # seq len
B = 4             # batch
HD = 64           # head dim
KT = 8            # d_model k-tiles of 128
NTT = 16          # token tiles of 128
NTB = 4           # token blocks of 512
NPAIR = 4         # head pairs per core (8 heads)
VSTR = 8 * 65     # v_aug cols per token tile (8 heads x 65)
SCALE = 1.0 / np.sqrt(HD)
WARMUP_MM = 16

_CACHE = {}


def _build_program():
    import concourse.mybir as mybir
    import concourse.tile as tile
    from concourse import bacc

    dt = mybir.dt
    f32, f32r, bf16 = dt.float32, dt.float32r, dt.bfloat16
    AF = mybir.ActivationFunctionType

    nc = bacc.Bacc("TRN2", target_bir_lowering=False, debug=False,
                   enable_asserts=False, num_devices=8)

    xT_d = nc.dram_tensor("xT", [D, T], f32, kind="ExternalInput").ap()
    wqkT_d = nc.dram_tensor("wqkT", [D, 1024], f32, kind="ExternalInput").ap()
    wvT_d = nc.dram_tensor("wvT", [D, 512], f32, kind="ExternalInput").ap()
    bqk_d = nc.dram_tensor("bqk", [128, 8], f32, kind="ExternalInput").ap()
    bv_d = nc.dram_tensor("bv", [128, 512], f32, kind="ExternalInput").ap()
    woT_d = nc.dram_tensor("woT", [512, 1024], bf16, kind="ExternalInput").ap()
    mask_d = nc.dram_tensor("mask2", [128, 256], bf16, kind="ExternalInput").ap()
    outT_d = nc.dram_tensor("outT", [D, T], f32, kind="ExternalOutput").ap()
    warm_d = nc.dram_tensor("warm", [1, 512], f32, kind="ExternalOutput").ap()

    with tile.TileContext(nc) as tc:
        with tc.tile_pool(name="const", bufs=1) as cpool, \
             tc.tile_pool(name="xt", bufs=1) as xpool, \
             tc.tile_pool(name="wqk", bufs=3) as wqkpool, \
             tc.tile_pool(name="qk", bufs=4) as qkpool, \
             tc.tile_pool(name="vt", bufs=1) as vpool, \
             tc.tile_pool(name="exp", bufs=3) as epool, \
             tc.tile_pool(name="at", bufs=1) as apool, \
             tc.tile_pool(name="rcp", bufs=1) as rpool, \
             tc.tile_pool(name="rbc", bufs=2) as rbpool, \
             tc.tile_pool(name="stg", bufs=2) as spool, \
             tc.tile_pool(name="big", bufs=2, space="PSUM") as pp_big, \
             tc.tile_pool(name="pv", bufs=2, space="PSUM") as pp_pv:

            # ---- PE warm-up: keep the clock un-throttled during DMA ----
            wtile = cpool.tile([128, 512], f32, tag="wrm")
            nc.vector.memset(wtile[:], 0.001)
            wps = pp_big.tile([128, 1024], f32, tag="big")
            for i in range(WARMUP_MM):
                nc.tensor.matmul(wps[:, 0:512], wtile[:, 0:128], wtile[:],
                                 start=(i == 0), stop=(i == WARMUP_MM - 1))
            wout = cpool.tile([1, 512], f32, tag="wout")
            nc.vector.tensor_copy(wout[:], wps[0:1, 0:512])
            nc.sync.dma_start(warm_d, wout[:])

            # ---- constant loads (ordered: small + first-needed first) ----
            bqk_sb = cpool.tile([128, 8], f32, tag="bqk")
            nc.sync.dma_start(bqk_sb[:], bqk_d)
            bv_sb = cpool.tile([128, 512], f32, tag="bv")
            nc.sync.dma_start(bv_sb[:], bv_d)
            mask_sb = cpool.tile([128, 256], bf16, tag="mask")
            nc.sync.dma_start(mask_sb[:], mask_d)
            wvT_sb = cpool.tile([128, KT * 512], f32r, tag="wv")
            for kt in range(KT):
                nc.sync.dma_start(
                    wvT_sb[:, kt * 512:(kt + 1) * 512],
                    wvT_d[kt * 128:(kt + 1) * 128, :].bitcast(f32r))
            # xT chunked [kt, nb], ordered nb-major so early token blocks land first
            xT_sb = xpool.tile([128, KT * T], f32r)
            for nb in range(NTB):
                for kt in range(KT):
                    nc.sync.dma_start(
                        xT_sb[:, kt * T + nb * 512: kt * T + (nb + 1) * 512],
                        xT_d[kt * 128:(kt + 1) * 128,
                             nb * 512:(nb + 1) * 512].bitcast(f32r))
            woT_sb = cpool.tile([128, 4 * 1024], bf16, tag="wo")
            nc.sync.dma_start(
                woT_sb[:].rearrange("p (k f) -> p k f", k=4),
                woT_d.rearrange("(k p) f -> p k f", p=128))

            # ---- v phase: v_aug [token, head*65] bf16 (65th col = ones) ----
            v_sb = vpool.tile([128, NTT * VSTR], bf16)
            nc.vector.memset(
                v_sb[:].rearrange("p (g e) -> p g e", e=65)[:, :, 64:65], 1.0)
            for tt2 in range(NTT // 2):
                ps = pp_big.tile([128, 1024], f32, tag="big")
                for kt in range(KT):
                    for u in range(2):
                        tt = 2 * tt2 + u
                        nc.tensor.matmul(
                            ps[:, u * 512:(u + 1) * 512],
                            xT_sb[:, kt * T + tt * 128: kt * T + (tt + 1) * 128],
                            wvT_sb[:, kt * 512:(kt + 1) * 512],
                            start=(kt == 0), stop=(kt == KT - 1))
                nc.vector.tensor_add(
                    v_sb[:].rearrange("p (t h e) -> p t h e", t=NTT, h=8)
                        [:, 2 * tt2:2 * tt2 + 2, :, 0:64],
                    ps[:].rearrange("p (u h f) -> p u h f", u=2, h=8),
                    bv_sb[:].rearrange("p (u h f) -> p u h f", u=1, h=8)
                         .to_broadcast([128, 2, 8, 64]))

            # ---- per head-pair: qkT production + attention ----
            attn_sb = apool.tile([128, NPAIR * T], bf16)

            def emit_outproj(tbs, mds):
                for md in mds:
                    ps = pp_big.tile([128, 1024], f32, tag="big")
                    for kf in range(4):
                        for u, tb in enumerate(tbs):
                            nc.tensor.matmul(
                                ps[:, u * 512:(u + 1) * 512],
                                woT_sb[:, kf * 1024 + md * 128:
                                       kf * 1024 + (md + 1) * 128],
                                attn_sb[:, kf * T + tb * 512:
                                        kf * T + tb * 512 + 512],
                                start=(kf == 0), stop=(kf == 3))
                    st = spool.tile([128, 1024], f32, tag="stg")
                    w = 512 * len(tbs)
                    nc.vector.tensor_copy(st[0:128, 0:w], ps[0:128, 0:w])
                    nc.sync.dma_start(
                        outT_d[md * 128:(md + 1) * 128,
                               tbs[0] * 512: tbs[0] * 512 + w], st[0:128, 0:w])

            for pair in range(NPAIR):
                qk_tiles = {}
                for sel, m in (("q", pair), ("k", NPAIR + pair)):
                    w_m = wqkpool.tile([128, KT * 128], f32r, tag="wqk")
                    nc.sync.dma_start(
                        w_m[:].rearrange("p (k f) -> p k f", k=KT),
                        wqkT_d[:, m * 128:(m + 1) * 128]
                            .rearrange("(k p) f -> p k f", p=128)
                            .bitcast(f32r))
                    qk_t = qkpool.tile([128, T], f32r, tag="qk")
                    for nb2 in range(NTB // 2):
                        ps = pp_big.tile([128, 1024], f32, tag="big")
                        for kt in range(KT):
                            for u in range(2):
                                nb = 2 * nb2 + u
                                nc.tensor.matmul(
                                    ps[:, u * 512:(u + 1) * 512],
                                    w_m[:, kt * 128:(kt + 1) * 128],
                                    xT_sb[:, kt * T + nb * 512:
                                          kt * T + nb * 512 + 512],
                                    start=(kt == 0), stop=(kt == KT - 1))
                        nc.vector.tensor_scalar_add(
                            qk_t[:, nb2 * 1024:(nb2 + 1) * 1024], ps[:],
                            bqk_sb[:, m:m + 1])
                    qk_tiles[sel] = qk_t
                qT, kT_t = qk_tiles["q"], qk_tiles["k"]

                for qb in range(NTB):
                    pv = pp_pv.tile([65, 1024], f32, tag="pv")
                    nkt = 4 * qb + 4
                    for kt in range(nkt):
                        diag = kt - 4 * qb
                        off = max(diag, 0) * 128
                        sc = pp_big.tile([128, 1024], f32, tag="big")
                        for hh in range(2):
                            nc.tensor.matmul(
                                sc[:, hh * 512:(hh + 1) * 512],
                                kT_t[hh * 64:(hh + 1) * 64,
                                     kt * 128:(kt + 1) * 128],
                                qT[hh * 64:(hh + 1) * 64,
                                   qb * 512:(qb + 1) * 512],
                                start=True, stop=True,
                                tile_position=(hh * 64, 0))
                        et = epool.tile([128, 1024], bf16, tag="exp")
                        et3 = et[:].rearrange("p (h c) -> p h c", h=2)
                        sc3 = sc[:].rearrange("p (h c) -> p h c", h=2)
                        if off > 0:
                            nc.vector.memset(et3[:, :, 0:off], 0.0)
                        nc.scalar.activation(
                            et3[:, :, off:512], sc3[:, :, off:512],
                            AF.Exp, scale=float(SCALE))
                        if diag >= 0:
                            nc.vector.tensor_mul(
                                et3[:, :, off:off + 128],
                                et3[:, :, off:off + 128],
                                mask_sb[:].rearrange("p (h c) -> p h c", h=2))
                        for hh in range(2):
                            nc.tensor.matmul(
                                pv[:, hh * 512:(hh + 1) * 512],
                                v_sb[:, kt * VSTR + (2 * pair + hh) * 65:
                                     kt * VSTR + (2 * pair + hh) * 65 + 65],
                                et[:, hh * 512:(hh + 1) * 512],
                                start=(kt == 0), stop=(kt == nkt - 1))
                    den = rpool.tile([1, 1024], f32, tag="den")
                    nc.vector.tensor_copy(den[:], pv[64:65, :])
                    rc = rpool.tile([1, 1024], f32, tag="rc")
                    nc.vector.reciprocal_approx_fast(rc[:], den[:])
                    rb = rbpool.tile([64, 1024], f32, tag="rb")
                    nc.gpsimd.partition_broadcast(rb[:], rc[:])
                    for hh in range(2):
                        nc.vector.tensor_mul(
                            attn_sb[hh * 64:(hh + 1) * 64,
                                    pair * T + qb * 512: pair * T + qb * 512 + 512],
                            pv[0:64, hh * 512:(hh + 1) * 512],
                            rb[:, hh * 512:(hh + 1) * 512])

            # ---- output projection (bf16) ----
            for tb2 in range(NTB // 2):
                emit_outproj((2 * tb2, 2 * tb2 + 1), range(8))

    nc.compile()
    return nc


def _get_program():
    if "nc" not in _CACHE:
        _CACHE["nc"] = _build_program()
    return _CACHE["nc"]


def _make_core_inputs(x, w_qkv, b_qkv, w_out):
    mask = np.triu(np.ones((128, 128), np.float32))
    mask2 = np.concatenate([mask, mask], axis=1).astype(ml_dtypes.bfloat16)
    ins = []
    for c in range(8):
        b, hg = c // 2, c % 2
        h0 = hg * 512
        qsel = slice(h0, h0 + 512)
        ksel = slice(D + h0, D + h0 + 512)
        vsel = slice(2 * D + h0, 2 * D + h0 + 512)
        ins.append({
            "xT": np.ascontiguousarray(x[b].T),
            "wqkT": np.ascontiguousarray(
                np.concatenate([w_qkv[qsel], w_qkv[ksel]], axis=0).T),
            "wvT": np.ascontiguousarray(w_qkv[vsel].T),
            "bqk": np.ascontiguousarray(
                np.concatenate([b_qkv[qsel], b_qkv[ksel]]).reshape(8, 128).T),
            "bv": np.ascontiguousarray(
                np.broadcast_to(b_qkv[vsel], (128, 512))),
            "woT": np.ascontiguousarray(
                w_out[:, h0:h0 + 512].T).astype(ml_dtypes.bfloat16),
            "mask2": mask2,
        })
    return ins


def kernel(x, w_qkv, b_qkv, w_out, b_out, _trace=False):
    from concourse.bass_utils import run_bass_kernel_spmd

    x = np.asarray(x, np.float32)
    w_qkv = np.asarray(w_qkv, np.float32)
    b_qkv = np.asarray(b_qkv, np.float32)
    w_out = np.asarray(w_out, np.float32)
    b_out = np.asarray(b_out, np.float32)

    nc = _get_program()
    ins = _make_core_inputs(x, w_qkv, b_qkv, w_out)
    res = run_bass_kernel_spmd(nc, ins, core_ids=list(range(8)), trace=_trace)
    _CACHE["last_result"] = res

    out = np.empty((B, T, D), np.float32)
    for b in range(B):
        s = res.results[2 * b]["outT"] + res.results[2 * b + 1]["outT"]
        out[b] = s.T + b_out
    return out
